# revision 24
# baseline (speedup 1.0000x reference)
"""BiLSTM (2-layer, bidirectional) Trainium2 Bass kernel.

Model: V=512, E=256, H=512, B=64, T=512.
  logits = FC(concat(fwd_stack(emb[x]), rev(bwd_stack(emb[x][::-1]))))

Distribution over 8 NeuronCores (one SPMD program; per-core data differs):
  rank 0..3 = layer-0 of chains (F,b0), (F,b1), (B,b0), (B,b1)  [b: batch half]
  rank 4..7 = layer-1 of the same chains.
Layer-0 ranks compute the embedding + layer-0 recurrence and ship hidden
states (pre-transposed) to the matching layer-1 rank via per-16-step
AllGathers.  Layer-1 ranks run LAG=48 iterations behind (a per-step 0/1 mask
zeroes their state until real data arrives) and end with the direction's half
of the FC; the host adds the two halves + fc bias.

Per iteration (batch 32): gate pre-activations z accumulate into one PSUM bank
through 9 moving-operand waves x 4 column-tiled PE groups (the four 32-column
PE quarters compute gate quarters i,f,o,g concurrently from their own weight
streams); sigmoid/tanh on ScalarE; c/h updates on VectorE; h re-transposed on
the PE into the next step's stationary layout.
"""

import os
import sys

for _p in ("/opt/trn_rl_repo", "/root/.axon_site/_ro/trn_rl_repo"):
    if os.path.isdir(_p) and _p not in sys.path:
        sys.path.insert(0, _p)

import numpy as np

import concourse.bacc as bacc
import concourse.mybir as mybir
import concourse.tile as tile
from concourse import bass
from concourse.bass_utils import run_bass_kernel_spmd
from concourse.masks import make_identity

F16 = mybir.dt.float16
F32 = mybir.dt.float32
EQ = mybir.AluOpType.is_equal
MUL = mybir.AluOpType.mult
ADD = mybir.AluOpType.add

V, E, H, B = 512, 256, 512, 64
T = int(os.environ.get("BILSTM_T", "512"))   # dev knob; grading uses 512
BSH = 32                      # batch per core
G4 = 4 * H                    # 2048 gate columns
LAG = 48                      # layer-1 data lag (3 blocks)
BLK = 16                      # steps per AllGather block
N_ITER = T + LAG              # 560
N_BLK = N_ITER // BLK         # 35
N_AG = N_BLK - LAG // BLK     # 32: AG k feeds ich slots of delivery d=k+3
NTOK = T * BSH                # 16384 token slots in ich (mod-T ring)
TOKBLK = 512                  # embedding bulk token tile
NO_AG = bool(int(os.environ.get("BILSTM_NO_AG", "0")))    # ablation knobs
NO_H1D = bool(int(os.environ.get("BILSTM_NO_H1D", "0")))
NO_ELEM = bool(int(os.environ.get("BILSTM_NO_ELEM", "0")))

_CACHE = {}


def _build():
    if "nc" in _CACHE:
        return _CACHE["nc"]
    nc = bacc.Bacc("TRN2", target_bir_lowering=False, debug=False, num_devices=8)

    x_in = nc.dram_tensor("x_ids", [1, NTOK], F16, kind="ExternalInput")
    emb_in = nc.dram_tensor("embt", [V, E], F16, kind="ExternalInput")
    wi_in = nc.dram_tensor("wiT", [4, 128, G4], F16, kind="ExternalInput")
    wh_in = nc.dram_tensor("whT", [4, 128, G4], F16, kind="ExternalInput")
    b_in = nc.dram_tensor("bias", [1, 2, G4], F16, kind="ExternalInput")
    wfc_in = nc.dram_tensor("wfcT", [4, 128, V], F16, kind="ExternalInput")
    mr_in = nc.dram_tensor("maskR", [128, 1], F16, kind="ExternalInput")
    log_out = nc.dram_tensor("logits", [N_ITER, BSH, V], F16, kind="ExternalOutput")

    with tile.TileContext(nc) as tc:
        with tc.tile_pool(name="per", bufs=1) as per, \
             tc.tile_pool(name="dramp", bufs=1, space="DRAM") as dram:

            # ---------------- persistent tiles ----------------
            wiT = per.tile([128, 4, G4], F16)
            whT = per.tile([128, 4, G4], F16)
            wfcT = per.tile([128, 4, V], F16)
            bias = per.tile([1, 2, G4], F16)        # row 0: warmup (0 for L1)
            embt = per.tile([128, 4, E], F16)       # [p, vchunk, e]
            maskR = per.tile([128, 1], F16)
            ones32 = per.tile([1, 32], F16)
            ident = per.tile([32, 32], F16)
            iotac = per.tile([128, 4], F32)         # p + 128*c
            ich = per.tile([128, 4, T, BSH], F16)   # input-chunk ring [p, kc, t, b]
            hist = per.tile([128, 4, BLK, BSH], F16)  # hT ring [p, kc, slot, b]
            stc = per.tile([64, 512], F16)          # c state lives at rows 32:64
            h1d = dram.tile([4, 128, N_ITER, BSH], F16)  # hT history (for FC tail)

            nc.sync.dma_start(wiT, wi_in.ap().rearrange("c p n -> p c n"))
            nc.sync.dma_start(whT, wh_in.ap().rearrange("c p n -> p c n"))
            nc.sync.dma_start(wfcT, wfc_in.ap().rearrange("c p n -> p c n"))
            nc.sync.dma_start(bias, b_in[:, :, :])
            nc.sync.dma_start(embt, emb_in.ap().rearrange("(c p) e -> p c e", p=128))
            nc.sync.dma_start(maskR, mr_in[:, :])
            nc.vector.memset(ones32, 1.0)
            make_identity(nc, ident)
            nc.gpsimd.iota(iotac[:, 0:1], [[1, 1]], channel_multiplier=1,
                           allow_small_or_imprecise_dtypes=True)
            for c in range(1, 4):
                nc.vector.tensor_scalar_add(iotac[:, c:c + 1], iotac[:, 0:1],
                                            128.0 * c)
            for _e in range(4):
                nc.vector.memset(ich[:, _e, :, :], 0.0)
            nc.vector.memset(hist, 0.0)
            nc.vector.memset(stc, 0.0)

            # ---------------- P1: embedding bulk ----------------
            # ich[:, 0:2, t, b] = emb[x[t*BSH+b]].T  (fp16 one-hot matmul)
            with tc.tile_pool(name="embp", bufs=2) as ep_sb, \
                 tc.tile_pool(name="embps", bufs=2, space="PSUM") as ep_ps:
                for blk in range(NTOK // TOKBLK):
                    xr = ep_sb.tile([128, TOKBLK], F16, tag="xr")
                    nc.sync.dma_start(
                        xr, bass.AP(tensor=x_in, offset=blk * TOKBLK,
                                    ap=[[0, 128], [1, TOKBLK]]))
                    ohs = []
                    for c in range(4):
                        oh = ep_sb.tile([128, TOKBLK], F16, tag=f"oh{c}")
                        nc.vector.tensor_scalar(
                            out=oh, in0=xr, scalar1=iotac[:, c:c + 1],
                            scalar2=None, op0=EQ)
                        ohs.append(oh)
                    t0 = blk * TOKBLK // BSH
                    nsl = TOKBLK // BSH
                    for e in range(2):
                        ep = ep_ps.tile([128, TOKBLK], F32, tag="ep")
                        for c in range(4):
                            nc.tensor.matmul(
                                ep, embt[:, c, 128 * e:128 * (e + 1)], ohs[c],
                                start=(c == 0), stop=(c == 3))
                        nc.vector.tensor_copy(
                            ich[:, e, t0:t0 + nsl, :],
                            ep[:, :].rearrange("p (t b) -> p t b", b=BSH))

            # ---------------- P2: recurrence ----------------
            with tc.tile_pool(name="zp", bufs=2, space="PSUM") as zp, \
                 tc.tile_pool(name="gp", bufs=2) as gp, \
                 tc.tile_pool(name="tcp", bufs=2) as tcp, \
                 tc.tile_pool(name="tpp", bufs=2, space="PSUM") as tpp, \
                 tc.tile_pool(name="typ", bufs=2) as typ, \
                 tc.tile_pool(name="mp", bufs=2) as mp, \
                 tc.tile_pool(name="hp", bufs=2) as hp, \
                 tc.tile_pool(name="agp", bufs=2) as agp, \
                 tc.tile_pool(name="agd", bufs=2, space="DRAM") as agd:

                for j in range(N_ITER):
                    slot_r = (j - 1) % BLK
                    slot_w = j % BLK

                    # --- 9 waves x 4 col-groups of matmuls into z ---
                    z = zp.tile([128, 512], F32, tag="z")
                    sel = 0 if j < LAG else 1
                    for q in range(4):          # wave 0: bias
                        nc.tensor.matmul(
                            z[32 * q:32 * (q + 1), :], ones32,
                            bias[:, sel, 512 * q:512 * (q + 1)],
                            start=True, stop=False, tile_position=(0, 32 * q))
                    for kc in range(4):         # waves 1-4: input chunks
                        st = ich[:, kc, j % T, :]
                        for q in range(4):
                            nc.tensor.matmul(
                                z[32 * q:32 * (q + 1), :], st,
                                wiT[:, kc, 512 * q:512 * (q + 1)],
                                start=False, stop=False,
                                tile_position=(0, 32 * q))
                    for kc in range(4):         # waves 5-8: recurrent chunks
                        st = hist[:, kc, slot_r, :]
                        for q in range(4):
                            nc.tensor.matmul(
                                z[32 * q:32 * (q + 1), :], st,
                                whT[:, kc, 512 * q:512 * (q + 1)],
                                start=False, stop=(kc == 3),
                                tile_position=(0, 32 * q))

                    # --- gate nonlinearities ---
                    # quarters: 0:32 = i, 32:64 = f, 64:96 = o, 96:128 = g
                    ty = typ.tile([96, 512], F16, tag="ty")
                    nc.scalar.activation(ty, z[0:96, :],
                                         mybir.ActivationFunctionType.Sigmoid)
                    gg = gp.tile([32, 512], F16, tag="g")
                    nc.scalar.activation(gg, z[96:128, :],
                                         mybir.ActivationFunctionType.Tanh)

                    # --- c/h update (mask_t gates the state reset for L1) ---
                    m1 = mp.tile([64, 512], F16, tag="m1")
                    nc.vector.tensor_tensor(
                        out=m1[32:64, :], in0=ty[32:64, :], in1=stc[32:64, :],
                        op=MUL)
                    m2 = mp.tile([64, 512], F16, tag="m2")
                    nc.vector.tensor_tensor(
                        out=m2[32:64, :], in0=ty[0:32, :], in1=gg, op=MUL)
                    nc.vector.tensor_tensor(
                        out=stc[32:64, :], in0=m1[32:64, :], in1=m2[32:64, :],
                        op=ADD)
                    tcs = tcp.tile([96, 512], F16, tag="tc")
                    nc.scalar.activation(tcs[64:96, :], stc[32:64, :],
                                         mybir.ActivationFunctionType.Tanh)
                    h = hp.tile([32, 512], F16, tag="h")
                    nc.vector.tensor_tensor(out=h, in0=ty[64:96, :],
                                            in1=tcs[64:96, :], op=MUL)

                    # --- transpose h into next step's stationary layout ---
                    tp = tpp.tile([128, 4, 32], F16, tag="tp")
                    for c in range(4):
                        nc.tensor.transpose(
                            tp[:, c, :], h[:, 128 * c:128 * (c + 1)], ident)
                    nc.vector.tensor_copy(hist[:, :, slot_w, :], tp)

                    # --- stream hT out for the FC tail ---
                    if not NO_H1D:
                        nc.sync.dma_start(
                            h1d[:, :, j:j + 1, :].rearrange("c p t b -> p c t b"),
                            hist[:, :, slot_w:slot_w + 1, :])

                    # --- block boundary: AllGather + delivery ---
                    if slot_w == BLK - 1 and not NO_AG:
                        k = j // BLK
                        if k < N_AG:
                            agin = agd.tile([128, 4, BLK, BSH], F16, tag="agin")
                            agout = agd.tile([256, 4, BLK, BSH], F16,
                                             tag="agout")
                            nc.sync.dma_start(agin, hist)
                            nc.gpsimd.collective_compute(
                                "AllGather", mybir.AluOpType.bypass,
                                replica_groups=[[0, 4], [1, 5], [2, 6], [3, 7]],
                                ins=[agin.opt()], outs=[agout.opt()])
                            d = k + LAG // BLK
                            s0 = (d * BLK) % T
                            agsb = agp.tile([128, 4, BLK, BSH], F16, tag="agsb")
                            nc.sync.dma_start(agsb, agout[0:128, :, :, :])
                            nc.vector.scalar_tensor_tensor(
                                out=ich[:, :, s0:s0 + BLK, :], in0=agsb,
                                scalar=maskR, in1=ich[:, :, s0:s0 + BLK, :],
                                op0=MUL, op1=ADD)

            # ---------------- P3: FC tail ----------------
            with tc.tile_pool(name="fcs", bufs=3) as fcs_p, \
                 tc.tile_pool(name="fco", bufs=3) as fco_p, \
                 tc.tile_pool(name="fcps", bufs=2, space="PSUM") as fcps_p:
                for tt in range(N_ITER * BSH // 128):
                    fcs = fcs_p.tile([128, 4, 128], F16, tag="fcs")
                    nc.sync.dma_start(
                        fcs,
                        h1d[:, :, 4 * tt:4 * (tt + 1), :]
                        .rearrange("c p t b -> p c (t b)"))
                    fps = fcps_p.tile([128, 512], F32, tag="fps")
                    for c in range(4):
                        nc.tensor.matmul(fps, fcs[:, c, :], wfcT[:, c, :],
                                         start=(c == 0), stop=(c == 3))
                    fsb = fco_p.tile([128, 512], F16, tag="fsb")
                    nc.vector.tensor_copy(fsb, fps)
                    nc.sync.dma_start(
                        log_out[4 * tt:4 * (tt + 1), :, :]
                        .rearrange("t b v -> (t b) v"),
                        fsb)

    nc.finalize()
    _CACHE["nc"] = nc
    return nc


# ---------------------------------------------------------------------------
# Host side
# ---------------------------------------------------------------------------

def _gate_perm():
    # torch gate row order i,f,g,o -> our quarter order i,f,o,g
    p = np.arange(G4).reshape(4, H)
    return np.concatenate([p[0], p[1], p[3], p[2]])


def _prep_core(x_sh, rev, layer, wih, whh, b, wfc_half, emb):
    """Build the input dict for one core. x_sh: [BSH, T] int; wih: [2048, D]."""
    perm = _gate_perm()
    if rev:
        x_sh = x_sh[:, ::-1]
    if layer == 0:
        ids = x_sh.T.reshape(-1).astype(np.float16)          # t-major
        embt = emb.astype(np.float16)
        wi = np.zeros((512, G4), np.float32)
        wi[:E] = wih[perm].T                                  # [E, 2048] padded
        b2 = np.stack([b[perm], b[perm]])                     # active always
        mrv = 0.0
        wfcT = np.zeros((4, 128, V), np.float16)
    else:
        ids = np.full(NTOK, -1.0, np.float16)
        embt = np.zeros((V, E), np.float16)
        wi = wih[perm].T                                      # [512, 2048]
        b2 = np.stack([np.zeros(G4, np.float32), b[perm]])    # warmup row = 0
        mrv = 1.0
        wfcT = np.ascontiguousarray(
            wfc_half.T.reshape(4, 128, V)).astype(np.float16)
    wh = whh[perm].T                                          # [512, 2048]
    return {
        "x_ids": ids.reshape(1, NTOK),
        "embt": embt,
        "wiT": np.ascontiguousarray(wi.reshape(4, 128, G4)).astype(np.float16),
        "whT": np.ascontiguousarray(wh.reshape(4, 128, G4)).astype(np.float16),
        "bias": b2.reshape(1, 2, G4).astype(np.float16),
        "wfcT": wfcT,
        "maskR": np.full((128, 1), mrv, np.float16),
    }


def _host_prep(x, emb, wih_f0, whh_f0, b_f0, wih_f1, whh_f1, b_f1,
               wih_b0, whh_b0, b_b0, wih_b1, whh_b1, b_b1, wfc, bfc):
    x = np.asarray(x)
    args = [np.asarray(a).astype(np.float32) for a in
            (emb, wih_f0, whh_f0, b_f0, wih_f1, whh_f1, b_f1,
             wih_b0, whh_b0, b_b0, wih_b1, whh_b1, b_b1, wfc, bfc)]
    (emb, wih_f0, whh_f0, b_f0, wih_f1, whh_f1, b_f1,
     wih_b0, whh_b0, b_b0, wih_b1, whh_b1, b_b1, wfc, bfc) = args

    in_maps = []
    # ranks 0..3: layer 0 of (F,b0),(F,b1),(B,b0),(B,b1)
    for (rev, w0, h0, bb0) in ((0, wih_f0, whh_f0, b_f0),
                               (1, wih_b0, whh_b0, b_b0)):
        for beta in range(2):
            in_maps.append(_prep_core(
                x[32 * beta:32 * (beta + 1)], rev, 0, w0, h0, bb0, None, emb))
    # ranks 4..7: layer 1, matching chain order
    for (rev, w1, h1, bb1, half) in ((0, wih_f1, whh_f1, b_f1, wfc[:, :H]),
                                     (1, wih_b1, whh_b1, b_b1, wfc[:, H:])):
        for beta in range(2):
            in_maps.append(_prep_core(
                x[32 * beta:32 * (beta + 1)], rev, 1, w1, h1, bb1, half, emb))
    return in_maps


def _host_prep_only(inputs):
    return _host_prep(**inputs)


def kernel(**inputs):
    nc = _build()
    in_maps = _host_prep(**inputs)
    bfc = np.asarray(inputs["bfc"]).astype(np.float32)

    res = run_bass_kernel_spmd(nc, in_maps, core_ids=list(range(8)))
    outs = [r["logits"].astype(np.float32) for r in res.results]

    logits = np.zeros((B, T, V), np.float32)
    # rank 4,5 = forward layer-1 halves; 6,7 = backward (time-reversed)
    for beta in range(2):
        f = outs[4 + beta][LAG:, :, :]            # [T, 32, V]
        bwd = outs[6 + beta][LAG:, :, :][::-1]    # un-reverse
        logits[32 * beta:32 * (beta + 1)] = (
            f.transpose(1, 0, 2) + bwd.transpose(1, 0, 2))
    logits += bfc.reshape(1, 1, V)
    return logits


# revision 25
# speedup vs baseline: 1077.0840x; 1077.0840x over previous
"""BiLSTM (2-layer, bidirectional) Trainium2 Bass kernel.

Model: V=512, E=256, H=512, B=64, T=512.
  logits = FC(concat(fwd_stack(emb[x]), rev(bwd_stack(emb[x][::-1]))))

Distribution over 8 NeuronCores (one SPMD program; per-core data differs):
  rank 0..3 = layer-0 of chains (F,b0), (F,b1), (B,b0), (B,b1)  [b: batch half]
  rank 4..7 = layer-1 of the same chains.
Layer-0 ranks compute the embedding + layer-0 recurrence and ship hidden
states (pre-transposed) to the matching layer-1 rank via per-16-step
AllGathers.  Layer-1 ranks run LAG=48 iterations behind (a per-step 0/1 mask
zeroes their state until real data arrives) and end with the direction's half
of the FC; the host adds the two halves + fc bias.

Per iteration (batch 32): gate pre-activations z accumulate into one PSUM bank
through 9 moving-operand waves x 4 column-tiled PE groups (the four 32-column
PE quarters compute gate quarters i,f,o,g concurrently from their own weight
streams); sigmoid/tanh on ScalarE; c/h updates on VectorE; h re-transposed on
the PE into the next step's stationary layout.
"""

import os
import sys

for _p in ("/opt/trn_rl_repo", "/root/.axon_site/_ro/trn_rl_repo"):
    if os.path.isdir(_p) and _p not in sys.path:
        sys.path.insert(0, _p)

import numpy as np

import concourse.bacc as bacc
import concourse.mybir as mybir
import concourse.tile as tile
from concourse import bass
from concourse.bass_utils import run_bass_kernel_spmd
from concourse.masks import make_identity

F16 = mybir.dt.float16
F32 = mybir.dt.float32
EQ = mybir.AluOpType.is_equal
MUL = mybir.AluOpType.mult
ADD = mybir.AluOpType.add

V, E, H, B = 512, 256, 512, 64
T = int(os.environ.get("BILSTM_T", "512"))   # dev knob; grading uses 512
BSH = 32                      # batch per core
G4 = 4 * H                    # 2048 gate columns
LAG = 48                      # layer-1 data lag (3 blocks)
BLK = 16                      # steps per AllGather block
N_ITER = T + LAG              # 560
N_BLK = N_ITER // BLK         # 35
N_AG = N_BLK - LAG // BLK     # 32: AG k feeds ich slots of delivery d=k+3
NTOK = T * BSH                # 16384 token slots in ich (mod-T ring)
TOKBLK = 512                  # embedding bulk token tile
NO_AG = bool(int(os.environ.get("BILSTM_NO_AG", "0")))    # ablation knobs
NO_H1D = bool(int(os.environ.get("BILSTM_NO_H1D", "0")))
NO_ELEM = bool(int(os.environ.get("BILSTM_NO_ELEM", "0")))

_CACHE = {}


def _build():
    if "nc" in _CACHE:
        return _CACHE["nc"]
    nc = bacc.Bacc("TRN2", target_bir_lowering=False, debug=False, num_devices=8)

    x_in = nc.dram_tensor("x_ids", [1, NTOK], F16, kind="ExternalInput")
    emb_in = nc.dram_tensor("embt", [V, E], F16, kind="ExternalInput")
    wi_in = nc.dram_tensor("wiT", [4, 128, G4], F16, kind="ExternalInput")
    wh_in = nc.dram_tensor("whT", [4, 128, G4], F16, kind="ExternalInput")
    b_in = nc.dram_tensor("bias", [1, 2, G4], F16, kind="ExternalInput")
    wfc_in = nc.dram_tensor("wfcT", [4, 128, V], F16, kind="ExternalInput")
    mr_in = nc.dram_tensor("maskR", [128, 1], F16, kind="ExternalInput")
    log_out = nc.dram_tensor("logits", [N_ITER, BSH, V], F16, kind="ExternalOutput")

    with tile.TileContext(nc) as tc:
        with tc.tile_pool(name="per", bufs=1) as per, \
             tc.tile_pool(name="dramp", bufs=1, space="DRAM") as dram:

            # ---------------- persistent tiles ----------------
            wiT = per.tile([128, 4, G4], F16)
            whT = per.tile([128, 4, G4], F16)
            wfcT = per.tile([128, 4, V], F16)
            bias = per.tile([1, 2, G4], F16)        # row 0: warmup (0 for L1)
            embt = per.tile([128, 4, E], F16)       # [p, vchunk, e]
            maskR = per.tile([128, 1], F16)
            ones32 = per.tile([1, 32], F16)
            ident = per.tile([32, 32], F16)
            iotac = per.tile([128, 4], F32)         # p + 128*c
            ich = per.tile([128, 4, T, BSH], F16)   # input-chunk ring [p, kc, t, b]
            hist = per.tile([128, 4, BLK, BSH], F16)  # hT ring [p, kc, slot, b]
            stc = per.tile([64, 512], F16)          # c state lives at rows 32:64
            h1d = dram.tile([4, 128, N_ITER, BSH], F16)  # hT history (for FC tail)

            nc.sync.dma_start(wiT, wi_in.ap().rearrange("c p n -> p c n"))
            nc.sync.dma_start(whT, wh_in.ap().rearrange("c p n -> p c n"))
            nc.sync.dma_start(wfcT, wfc_in.ap().rearrange("c p n -> p c n"))
            nc.sync.dma_start(bias, b_in[:, :, :])
            nc.sync.dma_start(embt, emb_in.ap().rearrange("(c p) e -> p c e", p=128))
            nc.sync.dma_start(maskR, mr_in[:, :])
            nc.vector.memset(ones32, 1.0)
            make_identity(nc, ident)
            nc.gpsimd.iota(iotac[:, 0:1], [[1, 1]], channel_multiplier=1,
                           allow_small_or_imprecise_dtypes=True)
            for c in range(1, 4):
                nc.vector.tensor_scalar_add(iotac[:, c:c + 1], iotac[:, 0:1],
                                            128.0 * c)
            for _e in range(4):
                nc.vector.memset(ich[:, _e, :, :], 0.0)
            nc.vector.memset(hist, 0.0)
            nc.vector.memset(stc, 0.0)

            # ---------------- P1: embedding bulk ----------------
            # ich[:, 0:2, t, b] = emb[x[t*BSH+b]].T  (fp16 one-hot matmul)
            with tc.tile_pool(name="embp", bufs=2) as ep_sb, \
                 tc.tile_pool(name="embps", bufs=2, space="PSUM") as ep_ps:
                for blk in range(NTOK // TOKBLK):
                    xr = ep_sb.tile([128, TOKBLK], F16, tag="xr")
                    nc.sync.dma_start(
                        xr, bass.AP(tensor=x_in, offset=blk * TOKBLK,
                                    ap=[[0, 128], [1, TOKBLK]]))
                    ohs = []
                    for c in range(4):
                        oh = ep_sb.tile([128, TOKBLK], F16, tag=f"oh{c}")
                        nc.vector.tensor_scalar(
                            out=oh, in0=xr, scalar1=iotac[:, c:c + 1],
                            scalar2=None, op0=EQ)
                        ohs.append(oh)
                    t0 = blk * TOKBLK // BSH
                    nsl = TOKBLK // BSH
                    for e in range(2):
                        ep = ep_ps.tile([128, TOKBLK], F32, tag="ep")
                        for c in range(4):
                            nc.tensor.matmul(
                                ep, embt[:, c, 128 * e:128 * (e + 1)], ohs[c],
                                start=(c == 0), stop=(c == 3))
                        nc.vector.tensor_copy(
                            ich[:, e, t0:t0 + nsl, :],
                            ep[:, :].rearrange("p (t b) -> p t b", b=BSH))

            # ---------------- P2: recurrence ----------------
            with tc.tile_pool(name="zp", bufs=3, space="PSUM") as zp, \
                 tc.tile_pool(name="gp", bufs=3) as gp, \
                 tc.tile_pool(name="tcp", bufs=3) as tcp, \
                 tc.tile_pool(name="tpp", bufs=3, space="PSUM") as tpp, \
                 tc.tile_pool(name="typ", bufs=3) as typ, \
                 tc.tile_pool(name="mp", bufs=3) as mp, \
                 tc.tile_pool(name="hp", bufs=3) as hp, \
                 tc.tile_pool(name="agp", bufs=3) as agp, \
                 tc.tile_pool(name="agd", bufs=3, space="DRAM") as agd:

                for j in range(N_ITER):
                    slot_r = (j - 1) % BLK
                    slot_w = j % BLK

                    # --- 9 waves x 4 col-groups of matmuls into z ---
                    z = zp.tile([128, 512], F32, tag="z")
                    sel = 0 if j < LAG else 1
                    for q in range(4):          # wave 0: bias
                        nc.tensor.matmul(
                            z[32 * q:32 * (q + 1), :], ones32,
                            bias[:, sel, 512 * q:512 * (q + 1)],
                            start=True, stop=False, tile_position=(0, 32 * q))
                    for kc in range(4):         # waves 1-4: input chunks
                        st = ich[:, kc, j % T, :]
                        for q in range(4):
                            nc.tensor.matmul(
                                z[32 * q:32 * (q + 1), :], st,
                                wiT[:, kc, 512 * q:512 * (q + 1)],
                                start=False, stop=False,
                                tile_position=(0, 32 * q))
                    for kc in range(4):         # waves 5-8: recurrent chunks
                        st = hist[:, kc, slot_r, :]
                        for q in range(4):
                            nc.tensor.matmul(
                                z[32 * q:32 * (q + 1), :], st,
                                whT[:, kc, 512 * q:512 * (q + 1)],
                                start=False, stop=(kc == 3),
                                tile_position=(0, 32 * q))

                    # --- gate nonlinearities ---
                    # quarters: 0:32 = i, 32:64 = f, 64:96 = o, 96:128 = g
                    ty = typ.tile([96, 512], F16, tag="ty")
                    nc.scalar.activation(ty, z[0:96, :],
                                         mybir.ActivationFunctionType.Sigmoid)
                    gg = gp.tile([32, 512], F16, tag="g")
                    nc.scalar.activation(gg, z[96:128, :],
                                         mybir.ActivationFunctionType.Tanh)

                    # --- c/h update (mask_t gates the state reset for L1) ---
                    m1 = mp.tile([64, 512], F16, tag="m1")
                    nc.vector.tensor_tensor(
                        out=m1[32:64, :], in0=ty[32:64, :], in1=stc[32:64, :],
                        op=MUL)
                    m2 = mp.tile([64, 512], F16, tag="m2")
                    nc.vector.tensor_tensor(
                        out=m2[32:64, :], in0=ty[0:32, :], in1=gg, op=MUL)
                    nc.vector.tensor_tensor(
                        out=stc[32:64, :], in0=m1[32:64, :], in1=m2[32:64, :],
                        op=ADD)
                    tcs = tcp.tile([96, 512], F16, tag="tc")
                    nc.scalar.activation(tcs[64:96, :], stc[32:64, :],
                                         mybir.ActivationFunctionType.Tanh)
                    h = hp.tile([32, 512], F16, tag="h")
                    nc.vector.tensor_tensor(out=h, in0=ty[64:96, :],
                                            in1=tcs[64:96, :], op=MUL)

                    # --- transpose h into next step's stationary layout ---
                    tp = tpp.tile([128, 4, 32], F16, tag="tp")
                    for c in range(4):
                        nc.tensor.transpose(
                            tp[:, c, :], h[:, 128 * c:128 * (c + 1)], ident)
                    nc.vector.tensor_copy(hist[:, :, slot_w, :], tp)

                    # --- stream hT out for the FC tail ---
                    if not NO_H1D:
                        nc.sync.dma_start(
                            h1d[:, :, j:j + 1, :].rearrange("c p t b -> p c t b"),
                            hist[:, :, slot_w:slot_w + 1, :])

                    # --- block boundary: AllGather + delivery ---
                    if slot_w == BLK - 1 and not NO_AG:
                        k = j // BLK
                        if k < N_AG:
                            agin = agd.tile([128, 4, BLK, BSH], F16, tag="agin")
                            agout = agd.tile([256, 4, BLK, BSH], F16,
                                             tag="agout")
                            nc.sync.dma_start(agin, hist)
                            nc.gpsimd.collective_compute(
                                "AllGather", mybir.AluOpType.bypass,
                                replica_groups=[[0, 4], [1, 5], [2, 6], [3, 7]],
                                ins=[agin.opt()], outs=[agout.opt()])
                            d = k + LAG // BLK
                            s0 = (d * BLK) % T
                            agsb = agp.tile([128, 4, BLK, BSH], F16, tag="agsb")
                            nc.sync.dma_start(agsb, agout[0:128, :, :, :])
                            nc.vector.scalar_tensor_tensor(
                                out=ich[:, :, s0:s0 + BLK, :], in0=agsb,
                                scalar=maskR, in1=ich[:, :, s0:s0 + BLK, :],
                                op0=MUL, op1=ADD)

            # ---------------- P3: FC tail ----------------
            with tc.tile_pool(name="fcs", bufs=3) as fcs_p, \
                 tc.tile_pool(name="fco", bufs=3) as fco_p, \
                 tc.tile_pool(name="fcps", bufs=2, space="PSUM") as fcps_p:
                for tt in range(N_ITER * BSH // 128):
                    fcs = fcs_p.tile([128, 4, 128], F16, tag="fcs")
                    nc.sync.dma_start(
                        fcs,
                        h1d[:, :, 4 * tt:4 * (tt + 1), :]
                        .rearrange("c p t b -> p c (t b)"))
                    fps = fcps_p.tile([128, 512], F32, tag="fps")
                    for c in range(4):
                        nc.tensor.matmul(fps, fcs[:, c, :], wfcT[:, c, :],
                                         start=(c == 0), stop=(c == 3))
                    fsb = fco_p.tile([128, 512], F16, tag="fsb")
                    nc.vector.tensor_copy(fsb, fps)
                    nc.sync.dma_start(
                        log_out[4 * tt:4 * (tt + 1), :, :]
                        .rearrange("t b v -> (t b) v"),
                        fsb)

    nc.finalize()
    _CACHE["nc"] = nc
    return nc


# ---------------------------------------------------------------------------
# Host side
# ---------------------------------------------------------------------------

def _gate_perm():
    # torch gate row order i,f,g,o -> our quarter order i,f,o,g
    p = np.arange(G4).reshape(4, H)
    return np.concatenate([p[0], p[1], p[3], p[2]])


def _prep_core(x_sh, rev, layer, wih, whh, b, wfc_half, emb):
    """Build the input dict for one core. x_sh: [BSH, T] int; wih: [2048, D]."""
    perm = _gate_perm()
    if rev:
        x_sh = x_sh[:, ::-1]
    if layer == 0:
        ids = x_sh.T.reshape(-1).astype(np.float16)          # t-major
        embt = emb.astype(np.float16)
        wi = np.zeros((512, G4), np.float32)
        wi[:E] = wih[perm].T                                  # [E, 2048] padded
        b2 = np.stack([b[perm], b[perm]])                     # active always
        mrv = 0.0
        wfcT = np.zeros((4, 128, V), np.float16)
    else:
        ids = np.full(NTOK, -1.0, np.float16)
        embt = np.zeros((V, E), np.float16)
        wi = wih[perm].T                                      # [512, 2048]
        b2 = np.stack([np.zeros(G4, np.float32), b[perm]])    # warmup row = 0
        mrv = 1.0
        wfcT = np.ascontiguousarray(
            wfc_half.T.reshape(4, 128, V)).astype(np.float16)
    wh = whh[perm].T                                          # [512, 2048]
    return {
        "x_ids": ids.reshape(1, NTOK),
        "embt": embt,
        "wiT": np.ascontiguousarray(wi.reshape(4, 128, G4)).astype(np.float16),
        "whT": np.ascontiguousarray(wh.reshape(4, 128, G4)).astype(np.float16),
        "bias": b2.reshape(1, 2, G4).astype(np.float16),
        "wfcT": wfcT,
        "maskR": np.full((128, 1), mrv, np.float16),
    }


def _host_prep(x, emb, wih_f0, whh_f0, b_f0, wih_f1, whh_f1, b_f1,
               wih_b0, whh_b0, b_b0, wih_b1, whh_b1, b_b1, wfc, bfc):
    x = np.asarray(x)
    args = [np.asarray(a).astype(np.float32) for a in
            (emb, wih_f0, whh_f0, b_f0, wih_f1, whh_f1, b_f1,
             wih_b0, whh_b0, b_b0, wih_b1, whh_b1, b_b1, wfc, bfc)]
    (emb, wih_f0, whh_f0, b_f0, wih_f1, whh_f1, b_f1,
     wih_b0, whh_b0, b_b0, wih_b1, whh_b1, b_b1, wfc, bfc) = args

    in_maps = []
    # ranks 0..3: layer 0 of (F,b0),(F,b1),(B,b0),(B,b1)
    for (rev, w0, h0, bb0) in ((0, wih_f0, whh_f0, b_f0),
                               (1, wih_b0, whh_b0, b_b0)):
        for beta in range(2):
            in_maps.append(_prep_core(
                x[32 * beta:32 * (beta + 1)], rev, 0, w0, h0, bb0, None, emb))
    # ranks 4..7: layer 1, matching chain order
    for (rev, w1, h1, bb1, half) in ((0, wih_f1, whh_f1, b_f1, wfc[:, :H]),
                                     (1, wih_b1, whh_b1, b_b1, wfc[:, H:])):
        for beta in range(2):
            in_maps.append(_prep_core(
                x[32 * beta:32 * (beta + 1)], rev, 1, w1, h1, bb1, half, emb))
    return in_maps


def _host_prep_only(inputs):
    return _host_prep(**inputs)


def kernel(**inputs):
    nc = _build()
    in_maps = _host_prep(**inputs)
    bfc = np.asarray(inputs["bfc"]).astype(np.float32)

    res = run_bass_kernel_spmd(nc, in_maps, core_ids=list(range(8)))
    outs = [r["logits"].astype(np.float32) for r in res.results]

    logits = np.zeros((B, T, V), np.float32)
    # rank 4,5 = forward layer-1 halves; 6,7 = backward (time-reversed)
    for beta in range(2):
        f = outs[4 + beta][LAG:, :, :]            # [T, 32, V]
        bwd = outs[6 + beta][LAG:, :, :][::-1]    # un-reverse
        logits[32 * beta:32 * (beta + 1)] = (
            f.transpose(1, 0, 2) + bwd.transpose(1, 0, 2))
    logits += bfc.reshape(1, 1, V)
    return logits
